# revision 3
# baseline (speedup 1.0000x reference)
"""Correlation-network kernel for TRN2, batch-sharded over 8 NeuronCores.

Per core (one batch element b):
  A = feature_A[b] as [HW=2304, C=256], B = feature_B[b] likewise.
  out[m, n] = corr_raw[m, n] * s[n]
  where corr_raw = A @ B^T  and  s[n] = 1/sqrt(sum_m corr_raw[m, n]^2).
  The 1/C of the reference cancels between corr and penalty.

Column norms via the Gram chain: sum_m corr_raw[m,n]^2 = b_n^T (A^T A) b_n,
so G = A^T A ([256,256]) gives pen2 = colsum(B^T o (G B^T)) without a second
pass over the [2304,2304] output. The scale s is folded into B^T's columns so
the main GEMM directly emits scaled output.

v3 vs v2:
  - The whole norm chain runs in fp8e4m3 with MatmulPerfMode.DoubleRow
    (2 k-tiles per instruction, 0.5 cyc/row): G, Q = G B^T and the colsum
    drop from ~9.5us to ~2.4us of PE time. G is scaled by 1/256 in the
    psum->sbuf copy so Q/r stay inside fp8 range; the rsqrt activation's
    input scale of 256 compensates exactly. The norm chain only shapes the
    per-column scale s (~0.3% error budget); the main GEMM stays bf16.
  - a (natural-layout A) arrives as fp8 (it only feeds G) and bt8 is cast
    from bt on ACT instead of DMA'd: input bytes drop 3.54 -> 2.95 MB.
  - The s broadcast and the bts muls run on the otherwise-idle Pool engine
    (gpsimd); PE loses the old broadcast matmul, DVE loses the bts muls.
  - Panel drains are 1024 wide (two bank-aligned 512 matmul halves, one
    copy) to halve the per-op PSUM/SBUF access-latency tax, split ~6:7
    DVE:ACT to balance DVE's r muls against ACT's casts/rsqrts.
"""
import numpy as np

B, H, W, C = 8, 48, 48, 256
HW = H * W            # 2304
MT = HW // 128        # 18 m-tiles
T2 = MT // 2          # 9 paired m-tiles for DoubleRow G
CH_PIPE = [(0, 512), (512, 512), (1024, 512), (1536, 512), (2048, 256)]
CH_MM = [(0, 1024), (1024, 1024), (2048, 256)]

_CACHE = {}


def _build(reps=1):
    import concourse.bacc as bacc
    import concourse.mybir as mybir
    import concourse.tile as tile

    dt = mybir.dt
    f32 = dt.float32
    bf16 = dt.bfloat16
    fp8 = dt.float8e4
    DR = mybir.MatmulPerfMode.DoubleRow

    nc = bacc.Bacc(None, target_bir_lowering=False, debug=False)
    # Partition-major swizzled inputs (see marshal_inputs):
    #   a8 [p, (2*t+j)*C+c] = A[(2*t+j)*128+p, c]     (fp8, DoubleRow pairs)
    #   at [p, h*HW+n]      = A[n, h*128+p]           (A^T, GEMM lhsT source)
    #   bt [p, h*HW+n]      = B[n, h*128+p]           (B^T, chain + GEMM rhs)
    a8_dram = nc.dram_tensor("a8", [128, MT * C], fp8, kind="ExternalInput")
    at_dram = nc.dram_tensor("at", [128, 2 * HW], bf16, kind="ExternalInput")
    bt_dram = nc.dram_tensor("bt", [128, 2 * HW], bf16, kind="ExternalInput")
    o_dram = nc.dram_tensor("out", [HW, HW], bf16, kind="ExternalOutput")
    o_r = o_dram[:, :].rearrange("(t p) n -> p t n", p=128)

    with tile.TileContext(nc) as tc, nc.allow_low_precision(
            reason="bf16/fp8 pipeline is intentional; l2 tolerance is 2e-2"):
        consts = tc.alloc_tile_pool(name="consts", bufs=1)
        ones_f = consts.tile([128, 2, 1], f32)
        nc.vector.memset(ones_f, 1.0)
        ones8 = consts.tile([128, 2, 1], fp8)
        nc.vector.tensor_copy(ones8, ones_f)

        inp = tc.alloc_tile_pool(name="inp", bufs=2)
        sca = tc.alloc_tile_pool(name="sca", bufs=2)
        scr = tc.alloc_tile_pool(name="scr", bufs=3)
        panels = tc.alloc_tile_pool(name="panels", bufs=8)
        # PSUM: 3 x [128,1024] f32 GEMM slots (2 banks each) + 2 chain slots
        # [128,512] (pg / pq pair / pp rotate through them) = 8 banks.
        ps_gq = tc.alloc_tile_pool(name="ps_gq", bufs=2, space="PSUM")
        ps_mm = tc.alloc_tile_pool(name="ps_mm", bufs=3, space="PSUM")

        NCP = len(CH_PIPE)
        NCM = len(CH_MM)

        def make_tiles():
            a8 = inp.tile([128, T2, 2, C], fp8, tag="a8", name="a8")
            at = inp.tile([128, 2 * HW], bf16, tag="at", name="at")
            bt = inp.tile([128, 2 * HW], bf16, tag="bt", name="bt")
            bt8 = inp.tile([128, 2, HW], fp8, tag="bt8", name="bt8")
            g8 = sca.tile([128, 2, C], fp8, tag="g", name="g8")
            s_bf = sca.tile([1, HW], bf16, tag="s", name="s")
            sbc = sca.tile([128, HW], bf16, tag="sbc", name="sbc")
            bts = sca.tile([128, 2 * HW], bf16, tag="bts", name="bts")
            return dict(a8=a8, at=at, bt=bt, bt8=bt8, g8=g8, s_bf=s_bf,
                        sbc=sbc, bts=bts)

        def emit_dmas(tl):
            # a8 first: G gates the chain's critical path. Each partition
            # line >= 2KB keeps DMA at line rate (a8 4608B, halves 4608B).
            nc.sync.dma_start(out=tl["a8"], in_=a8_dram[:, :].rearrange(
                "p (t j c) -> p t j c", t=T2, j=2))
            for h in (0, 1):
                nc.sync.dma_start(out=tl["bt"][:, h * HW:(h + 1) * HW],
                                  in_=bt_dram[:, h * HW:(h + 1) * HW])
            nc.sync.dma_start(out=tl["at"], in_=at_dram[:, :])

        def emit_g(tl):
            # G = A^T A ([256,256]) in fp8 DoubleRow: 2 m-tiles per matmul,
            # both c-halves computed directly (Q's lhsT reads G[k, m] and G
            # is symmetric, so no transpose is needed either way).
            # g8[p, j, c'] = G[j*128+p, c'] / 256, scaled into fp8 range.
            a8, g8 = tl["a8"], tl["g8"]
            pg = ps_gq.tile([128, 512], f32, tag="pgq", name="pg")
            for j2 in (0, 1):
                for t in range(T2):
                    nc.tensor.matmul(
                        pg[:, j2 * C:(j2 + 1) * C],
                        a8[:, t, :, j2 * 128:(j2 + 1) * 128],
                        a8[:, t, :, :],
                        start=(t == 0), stop=(t == T2 - 1),
                        perf_mode=DR)
            for j in (0, 1):
                nc.scalar.activation(
                    g8[:, j, :], pg[:, j * C:(j + 1) * C],
                    mybir.ActivationFunctionType.Copy, scale=1.0 / 256)
            # bt8: fp8 cast of B^T for the DoubleRow chain (saves its DMA)
            for h in (0, 1):
                nc.scalar.copy(tl["bt8"][:, h, :],
                               tl["bt"][:, h * HW:(h + 1) * HW])

        def chunk_pipe(tl, ci):
            # pq_j = (G/256) B^T (one DoubleRow matmul per c-half);
            # r[:,j,:] = B^T o pq_j (fp8); pen2 = DoubleRow-colsum(r);
            # s = rsqrt(256 * pen2/256); sbc = broadcast s (Pool);
            # bts = B^T * s (Pool)
            bt, bt8, g8 = tl["bt"], tl["bt8"], tl["g8"]
            s_bf, sbc, bts = tl["s_bf"], tl["sbc"], tl["bts"]
            n0, cw = CH_PIPE[ci]
            r = scr.tile([128, 2, cw], fp8, tag=f"r{ci % 3}", name="r")
            for j in (0, 1):
                pq = ps_gq.tile([128, 512], f32, tag="pgq", name="pq")
                nc.tensor.matmul(
                    pq[:, :cw],
                    g8[:, :, j * 128:(j + 1) * 128],
                    bt8[:, :, n0:n0 + cw],
                    start=True, stop=True, perf_mode=DR)
                nc.vector.tensor_mul(
                    r[:, j, :], bt[:, j * HW + n0:j * HW + n0 + cw],
                    pq[:, :cw])
            pp = ps_gq.tile([128, 512], f32, tag="pgq", name="pp")
            nc.tensor.matmul(pp[0:1, :cw], ones8, r[:, :, :],
                             start=True, stop=True, perf_mode=DR)
            # s = 1/sqrt(pen2): ACT rsqrt, input scale 256 undoing the g8
            # scaling exactly (pen2 >= 0 so |x| = x).
            nc.scalar.activation(
                s_bf[:, n0:n0 + cw], pp[0:1, :cw],
                mybir.ActivationFunctionType.Abs_reciprocal_sqrt, scale=256.0)
            nc.gpsimd.partition_broadcast(sbc[:, n0:n0 + cw],
                                          s_bf[0:1, n0:n0 + cw])
            for h in (0, 1):
                nc.gpsimd.tensor_mul(
                    bts[:, h * HW + n0:h * HW + n0 + cw],
                    bt[:, h * HW + n0:h * HW + n0 + cw],
                    sbc[:, n0:n0 + cw])

        # Software pipeline across reps: rep r+1's input DMAs are issued at
        # wavefront step 4 of rep r (the in-order HWDGE queue reaches them
        # mid-body instead of after all of rep r's panel DMAs), and rep r+1's
        # G matmuls are emitted at step 12 (the PE FIFO reaches them when
        # a8 has long landed, so G runs gap-free inside rep r's stream).
        tiles = make_tiles()
        emit_dmas(tiles)
        emit_g(tiles)
        chunk_pipe(tiles, 0)
        chunk_pipe(tiles, 1)
        pipes_pre = False
        for _rep in range(reps):
            tl = tiles
            nxt = None

            # main GEMM on a diagonal wavefront: step k emits (mt, ci) with
            # mt = k - ci, so program order (= PE FIFO order) only ever needs
            # GEMM chunk ci ~k*2us after GEMM start; the chain chunks feeding
            # it are emitted one step ahead (rep 0 only -- later reps' chains
            # were pre-run inside the previous body, so their wavefronts
            # start with every bts chunk ready).
            at, bts = tl["at"], tl["bts"]
            panel_by_mt = {}
            for k in range(MT + NCM - 1):
                if not pipes_pre:
                    if k == 0:
                        chunk_pipe(tl, 2)
                        chunk_pipe(tl, 3)
                    elif k == 1:
                        chunk_pipe(tl, 4)
                if k == 4 and _rep + 1 < reps:
                    nxt = make_tiles()
                    emit_dmas(nxt)
                if k == 12 and nxt is not None:
                    emit_g(nxt)
                if nxt is not None and 14 <= k < 14 + NCP:
                    chunk_pipe(nxt, k - 14)
                for ci in range(NCM):
                    mt = k - ci
                    if not (0 <= mt < MT):
                        continue
                    n0, cw = CH_MM[ci]
                    if ci == 0:
                        panel_by_mt[mt] = panels.tile([128, HW], bf16,
                                                      tag="panel",
                                                      name="panel")
                    panel = panel_by_mt[mt]
                    # 3 pm slots (2 banks each): the wavefront keeps ~2 in
                    # flight; matmuls write bank-aligned 512 halves, the
                    # drain covers the whole 1024 in one op.
                    pm = ps_mm.tile([128, cw], f32, tag="pm", name="pm")
                    for q0 in range(0, cw, 512):
                        qw = min(512, cw - q0)
                        for h in (0, 1):
                            nc.tensor.matmul(
                                pm[:, q0:q0 + qw],
                                at[:, h * HW + mt * 128:h * HW + (mt + 1) * 128],
                                bts[:, h * HW + n0 + q0:h * HW + n0 + q0 + qw],
                                start=(h == 0), stop=(h == 1))
                    # ~6/13 of panel drains on DVE, rest on ACT: DVE also
                    # carries the chain's r muls, ACT the casts and rsqrts.
                    cp = (nc.vector.tensor_copy if (mt * 3 + ci) % 13 < 6
                          else nc.scalar.copy)
                    cp(panel[:, n0:n0 + cw], pm[:, :])
                    # split the panel store: the first piece fires two steps
                    # early, smoothing the write stream; both pieces keep
                    # partition lines >= 2KB for DMA line rate
                    if ci == 0:
                        nc.sync.dma_start(out=o_r[:, mt, :1024],
                                          in_=panel[:, :1024])
                    elif ci == NCM - 1:
                        nc.sync.dma_start(out=o_r[:, mt, 1024:],
                                          in_=panel[:, 1024:])
            if nxt is not None:
                tiles = nxt
                pipes_pre = True

        for pool in (ps_mm, ps_gq,
                     panels, scr, sca, inp, consts):
            pool.release()
    nc.finalize()
    return nc


def _get_nc(reps=1):
    key = ("nc", reps)
    if key not in _CACHE:
        _CACHE[key] = _build(reps)
    return _CACHE[key]


def marshal_inputs(feature_A, feature_B):
    """Full f32 inputs -> per-core partition-major bf16/fp8 arrays."""
    import ml_dtypes
    bf = ml_dtypes.bfloat16
    f8 = ml_dtypes.float8_e4m3
    fa = np.asarray(feature_A, dtype=np.float32).reshape(B, HW, C)
    fb = np.asarray(feature_B, dtype=np.float32).reshape(B, HW, C)
    # a8[b, p, (2*t+j)*C+c] = A[b, (2*t+j)*128+p, c]
    a8 = np.ascontiguousarray(
        fa.astype(f8).reshape(B, MT, 128, C).transpose(0, 2, 1, 3)
    ).reshape(B, 128, MT * C)
    # at[b, p, h*HW+n] = A[b, n, h*128+p]
    at_sw = np.ascontiguousarray(
        fa.astype(bf).reshape(B, HW, 2, 128).transpose(0, 3, 2, 1)
    ).reshape(B, 128, 2 * HW)
    bt_sw = np.ascontiguousarray(
        fb.astype(bf).reshape(B, HW, 2, 128).transpose(0, 3, 2, 1)
    ).reshape(B, 128, 2 * HW)
    return a8, at_sw, bt_sw


def run(feature_A, feature_B, trace=False):
    from concourse.bass_utils import run_bass_kernel_spmd

    nc = _get_nc()
    a8, at_sw, bt_sw = marshal_inputs(feature_A, feature_B)
    in_maps = [{"a8": a8[i], "at": at_sw[i], "bt": bt_sw[i]}
               for i in range(B)]
    res = run_bass_kernel_spmd(nc, in_maps, list(range(B)), trace=trace)
    out = np.stack([res.results[i]["out"].astype(np.float32)
                    for i in range(B)])
    return out.reshape(B, H, W, H, W), res


def kernel(feature_A, feature_B):
    out, _ = run(feature_A, feature_B)
    return out


# revision 5
# speedup vs baseline: 1.0182x; 1.0182x over previous
"""Correlation-network kernel for TRN2, batch-sharded over 8 NeuronCores.

Per core (one batch element b):
  A = feature_A[b] as [HW=2304, C=256], B = feature_B[b] likewise.
  out[m, n] = corr_raw[m, n] * s[n]
  where corr_raw = A @ B^T  and  s[n] = 1/sqrt(sum_m corr_raw[m, n]^2).
  The 1/C of the reference cancels between corr and penalty.

Column norms via the Gram chain: sum_m corr_raw[m,n]^2 = b_n^T (A^T A) b_n,
so G = A^T A ([256,256]) gives pen2 = colsum(B^T o (G B^T)) without a second
pass over the [2304,2304] output. The scale s is folded into B^T's columns so
the main GEMM directly emits scaled output.

v3 vs v2:
  - The whole norm chain runs in fp8e4m3 with MatmulPerfMode.DoubleRow
    (2 k-tiles per instruction, 0.5 cyc/row): G, Q = G B^T and the colsum
    drop from ~9.5us to ~2.4us of PE time. G is scaled by 1/256 in the
    psum->sbuf copy so Q/r stay inside fp8 range; the rsqrt activation's
    input scale of 256 compensates exactly. The norm chain only shapes the
    per-column scale s (~0.3% error budget); the main GEMM stays bf16.
  - a (natural-layout A) arrives as fp8 (it only feeds G) and bt8 is cast
    from bt on ACT instead of DMA'd: input bytes drop 3.54 -> 2.95 MB.
  - The s broadcast and the bts muls run on the otherwise-idle Pool engine
    (gpsimd); PE loses the old broadcast matmul, DVE loses the bts muls.
  - Panel drains are 1024 wide (two bank-aligned 512 matmul halves, one
    copy) to halve the per-op PSUM/SBUF access-latency tax, split ~6:7
    DVE:ACT to balance DVE's r muls against ACT's casts/rsqrts.
"""
import numpy as np

B, H, W, C = 8, 48, 48, 256
HW = H * W            # 2304
MT = HW // 128        # 18 m-tiles
T2 = MT // 2          # 9 paired m-tiles for DoubleRow G
CH_PIPE = [(0, 512), (512, 512), (1024, 512), (1536, 512), (2048, 256)]
CH_MM = [(0, 1024), (1024, 1024), (2048, 256)]

_CACHE = {}


def _build(reps=1):
    import concourse.bacc as bacc
    import concourse.mybir as mybir
    import concourse.tile as tile

    dt = mybir.dt
    f32 = dt.float32
    bf16 = dt.bfloat16
    fp8 = dt.float8e4
    DR = mybir.MatmulPerfMode.DoubleRow

    nc = bacc.Bacc(None, target_bir_lowering=False, debug=False)
    # Partition-major swizzled inputs (see marshal_inputs):
    #   a8 [p, (2*t+j)*C+c] = A[(2*t+j)*128+p, c]     (fp8, DoubleRow pairs)
    #   at [p, h*HW+n]      = A[n, h*128+p]           (A^T, GEMM lhsT source)
    #   bt [p, h*HW+n]      = B[n, h*128+p]           (B^T, chain + GEMM rhs)
    a8_dram = nc.dram_tensor("a8", [128, MT * C], fp8, kind="ExternalInput")
    at_dram = nc.dram_tensor("at", [128, 2 * HW], bf16, kind="ExternalInput")
    bt_dram = nc.dram_tensor("bt", [128, 2 * HW], bf16, kind="ExternalInput")
    o_dram = nc.dram_tensor("out", [HW, HW], bf16, kind="ExternalOutput")
    o_r = o_dram[:, :].rearrange("(t p) n -> p t n", p=128)

    with tile.TileContext(nc) as tc, nc.allow_low_precision(
            reason="bf16/fp8 pipeline is intentional; l2 tolerance is 2e-2"):
        consts = tc.alloc_tile_pool(name="consts", bufs=1)
        ones_f = consts.tile([128, 2, 1], f32)
        nc.vector.memset(ones_f, 1.0)
        ones8 = consts.tile([128, 2, 1], fp8)
        nc.vector.tensor_copy(ones8, ones_f)

        inp = tc.alloc_tile_pool(name="inp", bufs=2)
        sca = tc.alloc_tile_pool(name="sca", bufs=2)
        scr = tc.alloc_tile_pool(name="scr", bufs=3)
        panels = tc.alloc_tile_pool(name="panels", bufs=8)
        # PSUM: 3 x [128,1024] f32 GEMM slots (2 banks each) + 2 chain slots
        # [128,512] (pg / pq pair / pp rotate through them) = 8 banks.
        ps_gq = tc.alloc_tile_pool(name="ps_gq", bufs=2, space="PSUM")
        ps_mm = tc.alloc_tile_pool(name="ps_mm", bufs=3, space="PSUM")

        NCP = len(CH_PIPE)
        NCM = len(CH_MM)

        def make_tiles():
            a8 = inp.tile([128, T2, 2, C], fp8, tag="a8", name="a8")
            at = inp.tile([128, 2 * HW], bf16, tag="at", name="at")
            bt = inp.tile([128, 2 * HW], bf16, tag="bt", name="bt")
            bt8 = inp.tile([128, 2, HW], fp8, tag="bt8", name="bt8")
            g8 = sca.tile([128, 2, C], fp8, tag="g", name="g8")
            s_bf = sca.tile([1, HW], bf16, tag="s", name="s")
            sbc = sca.tile([128, HW], bf16, tag="sbc", name="sbc")
            bts = sca.tile([128, 2 * HW], bf16, tag="bts", name="bts")
            return dict(a8=a8, at=at, bt=bt, bt8=bt8, g8=g8, s_bf=s_bf,
                        sbc=sbc, bts=bts)

        def emit_dmas(tl):
            # a8 first: G gates the chain's critical path. Each partition
            # line >= 2KB keeps DMA at line rate (a8 4608B, halves 4608B).
            nc.sync.dma_start(out=tl["a8"], in_=a8_dram[:, :].rearrange(
                "p (t j c) -> p t j c", t=T2, j=2))
            for h in (0, 1):
                nc.sync.dma_start(out=tl["bt"][:, h * HW:(h + 1) * HW],
                                  in_=bt_dram[:, h * HW:(h + 1) * HW])
            nc.sync.dma_start(out=tl["at"], in_=at_dram[:, :])

        def emit_g(tl):
            # G = A^T A ([256,256]) in fp8 DoubleRow: 2 m-tiles per matmul,
            # both c-halves computed directly (Q's lhsT reads G[k, m] and G
            # is symmetric, so no transpose is needed either way).
            # g8[p, j, c'] = G[j*128+p, c'] / 256, scaled into fp8 range.
            a8, g8 = tl["a8"], tl["g8"]
            pg = ps_gq.tile([128, 512], f32, tag="pgq", name="pg")
            for j2 in (0, 1):
                for t in range(T2):
                    nc.tensor.matmul(
                        pg[:, j2 * C:(j2 + 1) * C],
                        a8[:, t, :, j2 * 128:(j2 + 1) * 128],
                        a8[:, t, :, :],
                        start=(t == 0), stop=(t == T2 - 1),
                        perf_mode=DR)
            for j in (0, 1):
                nc.scalar.activation(
                    g8[:, j, :], pg[:, j * C:(j + 1) * C],
                    mybir.ActivationFunctionType.Copy, scale=1.0 / 256)
            # bt8: fp8 cast of B^T for the DoubleRow chain (saves its DMA)
            for h in (0, 1):
                nc.scalar.copy(tl["bt8"][:, h, :],
                               tl["bt"][:, h * HW:(h + 1) * HW])

        def chunk_pipe(tl, ci):
            # pq_j = (G/256) B^T (one DoubleRow matmul per c-half);
            # r[:,j,:] = B^T o pq_j (fp8); pen2 = DoubleRow-colsum(r);
            # s = rsqrt(256 * pen2/256); sbc = broadcast s (Pool);
            # bts = B^T * s (Pool)
            bt, bt8, g8 = tl["bt"], tl["bt8"], tl["g8"]
            s_bf, sbc, bts = tl["s_bf"], tl["sbc"], tl["bts"]
            n0, cw = CH_PIPE[ci]
            r = scr.tile([128, 2, cw], fp8, tag=f"r{ci % 3}", name="r")
            for j in (0, 1):
                pq = ps_gq.tile([128, 512], f32, tag="pgq", name="pq")
                nc.tensor.matmul(
                    pq[:, :cw],
                    g8[:, :, j * 128:(j + 1) * 128],
                    bt8[:, :, n0:n0 + cw],
                    start=True, stop=True, perf_mode=DR)
                nc.vector.tensor_mul(
                    r[:, j, :], bt[:, j * HW + n0:j * HW + n0 + cw],
                    pq[:, :cw])
            pp = ps_gq.tile([128, 512], f32, tag="pgq", name="pp")
            nc.tensor.matmul(pp[0:1, :cw], ones8, r[:, :, :],
                             start=True, stop=True, perf_mode=DR)
            # s = 1/sqrt(pen2): ACT rsqrt, input scale 256 undoing the g8
            # scaling exactly (pen2 >= 0 so |x| = x).
            nc.scalar.activation(
                s_bf[:, n0:n0 + cw], pp[0:1, :cw],
                mybir.ActivationFunctionType.Abs_reciprocal_sqrt, scale=256.0)
            nc.gpsimd.partition_broadcast(sbc[:, n0:n0 + cw],
                                          s_bf[0:1, n0:n0 + cw])
            # all-bf16 muls hit DVE's 2x mode (0.52 ns/col)
            for h in (0, 1):
                nc.vector.tensor_mul(
                    bts[:, h * HW + n0:h * HW + n0 + cw],
                    bt[:, h * HW + n0:h * HW + n0 + cw],
                    sbc[:, n0:n0 + cw])

        # Software pipeline across reps: rep r+1's input DMAs are issued at
        # wavefront step 4 of rep r (the in-order HWDGE queue reaches them
        # mid-body instead of after all of rep r's panel DMAs), and rep r+1's
        # G matmuls are emitted at step 12 (the PE FIFO reaches them when
        # a8 has long landed, so G runs gap-free inside rep r's stream).
        tiles = make_tiles()
        emit_dmas(tiles)
        emit_g(tiles)
        chunk_pipe(tiles, 0)
        chunk_pipe(tiles, 1)
        pipes_pre = False
        for _rep in range(reps):
            tl = tiles
            nxt = None

            # main GEMM on a diagonal wavefront: step k emits (mt, ci) with
            # mt = k - ci, so program order (= PE FIFO order) only ever needs
            # GEMM chunk ci ~k*2us after GEMM start; the chain chunks feeding
            # it are emitted one step ahead (rep 0 only -- later reps' chains
            # were pre-run inside the previous body, so their wavefronts
            # start with every bts chunk ready).
            at, bts = tl["at"], tl["bts"]
            panel_by_mt = {}
            for k in range(MT + NCM - 1):
                if not pipes_pre:
                    if k == 0:
                        chunk_pipe(tl, 2)
                        chunk_pipe(tl, 3)
                    elif k == 1:
                        chunk_pipe(tl, 4)
                if k == 3 and _rep + 1 < reps:
                    nxt = make_tiles()
                    emit_dmas(nxt)
                if k == 8 and nxt is not None:
                    emit_g(nxt)
                if nxt is not None and 10 <= k < 10 + NCP:
                    chunk_pipe(nxt, k - 10)
                for ci in range(NCM):
                    mt = k - ci
                    if not (0 <= mt < MT):
                        continue
                    n0, cw = CH_MM[ci]
                    if ci == 0:
                        panel_by_mt[mt] = panels.tile([128, HW], bf16,
                                                      tag="panel",
                                                      name="panel")
                    panel = panel_by_mt[mt]
                    # 3 pm slots (2 banks each): the wavefront keeps ~2 in
                    # flight; matmuls write bank-aligned 512 halves, the
                    # drain covers the whole 1024 in one op.
                    pm = ps_mm.tile([128, cw], f32, tag="pm", name="pm")
                    for q0 in range(0, cw, 512):
                        qw = min(512, cw - q0)
                        for h in (0, 1):
                            nc.tensor.matmul(
                                pm[:, q0:q0 + qw],
                                at[:, h * HW + mt * 128:h * HW + (mt + 1) * 128],
                                bts[:, h * HW + n0 + q0:h * HW + n0 + q0 + qw],
                                start=(h == 0), stop=(h == 1))
                    # ~6/13 of panel drains on DVE, rest on ACT: DVE also
                    # carries the chain's r muls, ACT the casts and rsqrts.
                    cp = (nc.vector.tensor_copy if (mt * 3 + ci) % 13 < 6
                          else nc.scalar.copy)
                    cp(panel[:, n0:n0 + cw], pm[:, :])
                    # split the panel store: the first piece fires two steps
                    # early, smoothing the write stream; both pieces keep
                    # partition lines >= 2KB for DMA line rate
                    if ci == 0:
                        nc.sync.dma_start(out=o_r[:, mt, :1024],
                                          in_=panel[:, :1024])
                    elif ci == NCM - 1:
                        nc.sync.dma_start(out=o_r[:, mt, 1024:],
                                          in_=panel[:, 1024:])
            if nxt is not None:
                tiles = nxt
                pipes_pre = True

        for pool in (ps_mm, ps_gq,
                     panels, scr, sca, inp, consts):
            pool.release()
    nc.finalize()
    return nc


def _get_nc(reps=1):
    key = ("nc", reps)
    if key not in _CACHE:
        _CACHE[key] = _build(reps)
    return _CACHE[key]


def marshal_inputs(feature_A, feature_B):
    """Full f32 inputs -> per-core partition-major bf16/fp8 arrays."""
    import ml_dtypes
    bf = ml_dtypes.bfloat16
    f8 = ml_dtypes.float8_e4m3
    fa = np.asarray(feature_A, dtype=np.float32).reshape(B, HW, C)
    fb = np.asarray(feature_B, dtype=np.float32).reshape(B, HW, C)
    # a8[b, p, (2*t+j)*C+c] = A[b, (2*t+j)*128+p, c]
    a8 = np.ascontiguousarray(
        fa.astype(f8).reshape(B, MT, 128, C).transpose(0, 2, 1, 3)
    ).reshape(B, 128, MT * C)
    # at[b, p, h*HW+n] = A[b, n, h*128+p]
    at_sw = np.ascontiguousarray(
        fa.astype(bf).reshape(B, HW, 2, 128).transpose(0, 3, 2, 1)
    ).reshape(B, 128, 2 * HW)
    bt_sw = np.ascontiguousarray(
        fb.astype(bf).reshape(B, HW, 2, 128).transpose(0, 3, 2, 1)
    ).reshape(B, 128, 2 * HW)
    return a8, at_sw, bt_sw


def run(feature_A, feature_B, trace=False):
    from concourse.bass_utils import run_bass_kernel_spmd

    nc = _get_nc()
    a8, at_sw, bt_sw = marshal_inputs(feature_A, feature_B)
    in_maps = [{"a8": a8[i], "at": at_sw[i], "bt": bt_sw[i]}
               for i in range(B)]
    res = run_bass_kernel_spmd(nc, in_maps, list(range(B)), trace=trace)
    out = np.stack([res.results[i]["out"].astype(np.float32)
                    for i in range(B)])
    return out.reshape(B, H, W, H, W), res


def kernel(feature_A, feature_B):
    out, _ = run(feature_A, feature_B)
    return out


# revision 6
# speedup vs baseline: 2.1750x; 2.1362x over previous
"""Correlation-network kernel for TRN2, batch-sharded over 8 NeuronCores.

Per core (one batch element b):
  A = feature_A[b] as [HW=2304, C=256], B = feature_B[b] likewise.
  out[m, n] = corr_raw[m, n] * s[n]
  where corr_raw = A @ B^T  and  s[n] = 1/sqrt(sum_m corr_raw[m, n]^2).
  The 1/C of the reference cancels between corr and penalty.

Column norms via the Gram chain: sum_m corr_raw[m,n]^2 = b_n^T (A^T A) b_n,
so G = A^T A ([256,256]) gives pen2 = colsum(B^T o (G B^T)) without a second
pass over the [2304,2304] output. The scale s is folded into B^T's columns so
the main GEMM directly emits scaled output.

v3 vs v2:
  - The whole norm chain runs in fp8e4m3 with MatmulPerfMode.DoubleRow
    (2 k-tiles per instruction, 0.5 cyc/row): G, Q = G B^T and the colsum
    drop from ~9.5us to ~2.4us of PE time. G is scaled by 1/256 in the
    psum->sbuf copy so Q/r stay inside fp8 range; the rsqrt activation's
    input scale of 256 compensates exactly. The norm chain only shapes the
    per-column scale s (~0.3% error budget); the main GEMM stays bf16.
  - a (natural-layout A) arrives as fp8 (it only feeds G) and bt8 is cast
    from bt on ACT instead of DMA'd: input bytes drop 3.54 -> 2.95 MB.
  - The s broadcast and the bts muls run on the otherwise-idle Pool engine
    (gpsimd); PE loses the old broadcast matmul, DVE loses the bts muls.
  - Panel drains are 1024 wide (two bank-aligned 512 matmul halves, one
    copy) to halve the per-op PSUM/SBUF access-latency tax, split ~6:7
    DVE:ACT to balance DVE's r muls against ACT's casts/rsqrts.
"""
import numpy as np

B, H, W, C = 8, 48, 48, 256
HW = H * W            # 2304
MT = HW // 128        # 18 m-tiles
T2 = MT // 2          # 9 paired m-tiles for DoubleRow G
CH_PIPE = [(0, 512), (512, 512), (1024, 512), (1536, 512), (2048, 256)]
CH_MM = [(0, 1024), (1024, 1024), (2048, 256)]

_CACHE = {}


def _build(reps=1):
    import concourse.bacc as bacc
    import concourse.mybir as mybir
    import concourse.tile as tile

    dt = mybir.dt
    f32 = dt.float32
    bf16 = dt.bfloat16
    fp8 = dt.float8e4
    DR = mybir.MatmulPerfMode.DoubleRow

    nc = bacc.Bacc(None, target_bir_lowering=False, debug=False)
    # Partition-major swizzled inputs (see marshal_inputs):
    #   a8 [p, (2*t+j)*C+c] = A[(2*t+j)*128+p, c]     (fp8, DoubleRow pairs)
    #   at [p, h*HW+n]      = A[n, h*128+p]           (A^T, GEMM lhsT source)
    #   bt [p, h*HW+n]      = B[n, h*128+p]           (B^T, chain + GEMM rhs)
    a8_dram = nc.dram_tensor("a8", [128, MT * C], fp8, kind="ExternalInput")
    at_dram = nc.dram_tensor("at", [128, 2 * HW], bf16, kind="ExternalInput")
    bt_dram = nc.dram_tensor("bt", [128, 2 * HW], bf16, kind="ExternalInput")
    o_dram = nc.dram_tensor("out", [HW, HW], bf16, kind="ExternalOutput")
    o_r = o_dram[:, :].rearrange("(t p) n -> p t n", p=128)

    with tile.TileContext(nc) as tc, nc.allow_low_precision(
            reason="bf16/fp8 pipeline is intentional; l2 tolerance is 2e-2"):
        consts = tc.alloc_tile_pool(name="consts", bufs=1)
        # dual-fp8 ldweights needs >=32 weight columns: broadcast the
        # colsum into 32 identical psum rows and read row 0.
        ones_f = consts.tile([128, 2, 32], f32)
        nc.vector.memset(ones_f, 1.0)
        ones8 = consts.tile([128, 2, 32], fp8)
        nc.vector.tensor_copy(ones8, ones_f)

        inp = tc.alloc_tile_pool(name="inp", bufs=2)
        sca = tc.alloc_tile_pool(name="sca", bufs=2)
        scr = tc.alloc_tile_pool(name="scr", bufs=3)
        panels = tc.alloc_tile_pool(name="panels", bufs=8)
        # PSUM: 3 x [128,1024] f32 GEMM slots (2 banks each) + 2 chain slots
        # [128,512] (pg / pq pair / pp rotate through them) = 8 banks.
        ps_gq = tc.alloc_tile_pool(name="ps_gq", bufs=2, space="PSUM")
        ps_mm = tc.alloc_tile_pool(name="ps_mm", bufs=3, space="PSUM")

        NCP = len(CH_PIPE)
        NCM = len(CH_MM)

        def make_tiles():
            a8 = inp.tile([128, T2, 2, C], fp8, tag="a8", name="a8")
            at = inp.tile([128, 2 * HW], bf16, tag="at", name="at")
            bt = inp.tile([128, 2 * HW], bf16, tag="bt", name="bt")
            bt8 = inp.tile([128, 2, HW], fp8, tag="bt8", name="bt8")
            g8 = sca.tile([128, 2, C], fp8, tag="g", name="g8")
            s_bf = sca.tile([1, HW], bf16, tag="s", name="s")
            sbc = sca.tile([128, HW], bf16, tag="sbc", name="sbc")
            bts = sca.tile([128, 2 * HW], bf16, tag="bts", name="bts")
            return dict(a8=a8, at=at, bt=bt, bt8=bt8, g8=g8, s_bf=s_bf,
                        sbc=sbc, bts=bts)

        def emit_dmas(tl):
            # a8 first: G gates the chain's critical path. Each partition
            # line >= 2KB keeps DMA at line rate (a8 4608B, halves 4608B).
            nc.sync.dma_start(out=tl["a8"], in_=a8_dram[:, :].rearrange(
                "p (t j c) -> p t j c", t=T2, j=2))
            for h in (0, 1):
                nc.sync.dma_start(out=tl["bt"][:, h * HW:(h + 1) * HW],
                                  in_=bt_dram[:, h * HW:(h + 1) * HW])
            nc.sync.dma_start(out=tl["at"], in_=at_dram[:, :])

        def emit_g(tl):
            # G = A^T A ([256,256]) in fp8 DoubleRow: 2 m-tiles per matmul,
            # both c-halves computed directly (Q's lhsT reads G[k, m] and G
            # is symmetric, so no transpose is needed either way).
            # g8[p, j, c'] = G[j*128+p, c'] / 256, scaled into fp8 range.
            a8, g8 = tl["a8"], tl["g8"]
            pg = ps_gq.tile([128, 512], f32, tag="pgq", name="pg")
            for j2 in (0, 1):
                for t in range(T2):
                    nc.tensor.matmul(
                        pg[:, j2 * C:(j2 + 1) * C],
                        a8[:, t, :, j2 * 128:(j2 + 1) * 128],
                        a8[:, t, :, :],
                        start=(t == 0), stop=(t == T2 - 1),
                        perf_mode=DR)
            for j in (0, 1):
                nc.scalar.activation(
                    g8[:, j, :], pg[:, j * C:(j + 1) * C],
                    mybir.ActivationFunctionType.Copy, scale=1.0 / 256)
            # bt8: fp8 cast of B^T for the DoubleRow chain (saves its DMA)
            for h in (0, 1):
                nc.scalar.copy(tl["bt8"][:, h, :],
                               tl["bt"][:, h * HW:(h + 1) * HW])

        def chunk_pipe(tl, ci):
            # pq_j = (G/256) B^T (one DoubleRow matmul per c-half);
            # r[:,j,:] = B^T o pq_j (fp8); pen2 = DoubleRow-colsum(r);
            # s = rsqrt(256 * pen2/256); sbc = broadcast s (Pool);
            # bts = B^T * s (Pool)
            bt, bt8, g8 = tl["bt"], tl["bt8"], tl["g8"]
            s_bf, sbc, bts = tl["s_bf"], tl["sbc"], tl["bts"]
            n0, cw = CH_PIPE[ci]
            r = scr.tile([128, 2, cw], fp8, tag=f"r{ci % 3}", name="r")
            for j in (0, 1):
                pq = ps_gq.tile([128, 512], f32, tag="pgq", name="pq")
                nc.tensor.matmul(
                    pq[:, :cw],
                    g8[:, :, j * 128:(j + 1) * 128],
                    bt8[:, :, n0:n0 + cw],
                    start=True, stop=True, perf_mode=DR)
                nc.vector.tensor_mul(
                    r[:, j, :], bt[:, j * HW + n0:j * HW + n0 + cw],
                    pq[:, :cw])
            pp = ps_gq.tile([128, 512], f32, tag="pgq", name="pp")
            nc.tensor.matmul(pp[0:32, :cw], ones8, r[:, :, :],
                             start=True, stop=True, perf_mode=DR)
            # s = 1/sqrt(pen2): ACT rsqrt, input scale 256 undoing the g8
            # scaling exactly (pen2 >= 0 so |x| = x).
            nc.scalar.activation(
                s_bf[:, n0:n0 + cw], pp[0:1, :cw],
                mybir.ActivationFunctionType.Abs_reciprocal_sqrt, scale=256.0)
            nc.gpsimd.partition_broadcast(sbc[:, n0:n0 + cw],
                                          s_bf[0:1, n0:n0 + cw])
            # all-bf16 muls hit DVE's 2x mode (0.52 ns/col)
            for h in (0, 1):
                nc.vector.tensor_mul(
                    bts[:, h * HW + n0:h * HW + n0 + cw],
                    bt[:, h * HW + n0:h * HW + n0 + cw],
                    sbc[:, n0:n0 + cw])

        # Software pipeline across reps: rep r+1's input DMAs are issued at
        # wavefront step 4 of rep r (the in-order HWDGE queue reaches them
        # mid-body instead of after all of rep r's panel DMAs), and rep r+1's
        # G matmuls are emitted at step 12 (the PE FIFO reaches them when
        # a8 has long landed, so G runs gap-free inside rep r's stream).
        tiles = make_tiles()
        emit_dmas(tiles)
        emit_g(tiles)
        chunk_pipe(tiles, 0)
        chunk_pipe(tiles, 1)
        pipes_pre = False
        for _rep in range(reps):
            tl = tiles
            nxt = None

            # main GEMM on a diagonal wavefront: step k emits (mt, ci) with
            # mt = k - ci, so program order (= PE FIFO order) only ever needs
            # GEMM chunk ci ~k*2us after GEMM start; the chain chunks feeding
            # it are emitted one step ahead (rep 0 only -- later reps' chains
            # were pre-run inside the previous body, so their wavefronts
            # start with every bts chunk ready).
            at, bts = tl["at"], tl["bts"]
            panel_by_mt = {}
            for k in range(MT + NCM - 1):
                if not pipes_pre:
                    if k == 0:
                        chunk_pipe(tl, 2)
                        chunk_pipe(tl, 3)
                    elif k == 1:
                        chunk_pipe(tl, 4)
                if k == 3 and _rep + 1 < reps:
                    nxt = make_tiles()
                    emit_dmas(nxt)
                if k == 8 and nxt is not None:
                    emit_g(nxt)
                if nxt is not None and 10 <= k < 10 + NCP:
                    chunk_pipe(nxt, k - 10)
                for ci in range(NCM):
                    mt = k - ci
                    if not (0 <= mt < MT):
                        continue
                    n0, cw = CH_MM[ci]
                    if ci == 0:
                        panel_by_mt[mt] = panels.tile([128, HW], bf16,
                                                      tag="panel",
                                                      name="panel")
                    panel = panel_by_mt[mt]
                    # 3 pm slots (2 banks each): the wavefront keeps ~2 in
                    # flight; matmuls write bank-aligned 512 halves, the
                    # drain covers the whole 1024 in one op.
                    pm = ps_mm.tile([128, cw], f32, tag="pm", name="pm")
                    for q0 in range(0, cw, 512):
                        qw = min(512, cw - q0)
                        for h in (0, 1):
                            nc.tensor.matmul(
                                pm[:, q0:q0 + qw],
                                at[:, h * HW + mt * 128:h * HW + (mt + 1) * 128],
                                bts[:, h * HW + n0 + q0:h * HW + n0 + q0 + qw],
                                start=(h == 0), stop=(h == 1))
                    # ~6/13 of panel drains on DVE, rest on ACT: DVE also
                    # carries the chain's r muls, ACT the casts and rsqrts.
                    cp = (nc.vector.tensor_copy if (mt * 3 + ci) % 13 < 6
                          else nc.scalar.copy)
                    cp(panel[:, n0:n0 + cw], pm[:, :])
                    # split the panel store: the first piece fires two steps
                    # early, smoothing the write stream; both pieces keep
                    # partition lines >= 2KB for DMA line rate
                    if ci == 0:
                        nc.sync.dma_start(out=o_r[:, mt, :1024],
                                          in_=panel[:, :1024])
                    elif ci == NCM - 1:
                        nc.sync.dma_start(out=o_r[:, mt, 1024:],
                                          in_=panel[:, 1024:])
            if nxt is not None:
                tiles = nxt
                pipes_pre = True

        for pool in (ps_mm, ps_gq,
                     panels, scr, sca, inp, consts):
            pool.release()
    nc.finalize()
    return nc


def _get_nc(reps=1):
    key = ("nc", reps)
    if key not in _CACHE:
        _CACHE[key] = _build(reps)
    return _CACHE[key]


def marshal_inputs(feature_A, feature_B):
    """Full f32 inputs -> per-core partition-major bf16/fp8 arrays."""
    import ml_dtypes
    bf = ml_dtypes.bfloat16
    f8 = ml_dtypes.float8_e4m3
    fa = np.asarray(feature_A, dtype=np.float32).reshape(B, HW, C)
    fb = np.asarray(feature_B, dtype=np.float32).reshape(B, HW, C)
    # a8[b, p, (2*t+j)*C+c] = A[b, (2*t+j)*128+p, c]
    a8 = np.ascontiguousarray(
        fa.astype(f8).reshape(B, MT, 128, C).transpose(0, 2, 1, 3)
    ).reshape(B, 128, MT * C)
    # at[b, p, h*HW+n] = A[b, n, h*128+p]
    at_sw = np.ascontiguousarray(
        fa.astype(bf).reshape(B, HW, 2, 128).transpose(0, 3, 2, 1)
    ).reshape(B, 128, 2 * HW)
    bt_sw = np.ascontiguousarray(
        fb.astype(bf).reshape(B, HW, 2, 128).transpose(0, 3, 2, 1)
    ).reshape(B, 128, 2 * HW)
    return a8, at_sw, bt_sw


def run(feature_A, feature_B, trace=False):
    from concourse.bass_utils import run_bass_kernel_spmd

    nc = _get_nc()
    a8, at_sw, bt_sw = marshal_inputs(feature_A, feature_B)
    in_maps = [{"a8": a8[i], "at": at_sw[i], "bt": bt_sw[i]}
               for i in range(B)]
    res = run_bass_kernel_spmd(nc, in_maps, list(range(B)), trace=trace)
    out = np.stack([res.results[i]["out"].astype(np.float32)
                    for i in range(B)])
    return out.reshape(B, H, W, H, W), res


def kernel(feature_A, feature_B):
    out, _ = run(feature_A, feature_B)
    return out


# revision 7
# speedup vs baseline: 2.1845x; 1.0044x over previous
"""Correlation-network kernel for TRN2, batch-sharded over 8 NeuronCores.

Per core (one batch element b):
  A = feature_A[b] as [HW=2304, C=256], B = feature_B[b] likewise.
  out[m, n] = corr_raw[m, n] * s[n]
  where corr_raw = A @ B^T  and  s[n] = 1/sqrt(sum_m corr_raw[m, n]^2).
  The 1/C of the reference cancels between corr and penalty.

Column norms via the Gram chain: sum_m corr_raw[m,n]^2 = b_n^T (A^T A) b_n,
so G = A^T A ([256,256]) gives pen2 = colsum(B^T o (G B^T)) without a second
pass over the [2304,2304] output. The scale s is folded into B^T's columns so
the main GEMM directly emits scaled output.

v3 vs v2:
  - The whole norm chain runs in fp8e4m3 with MatmulPerfMode.DoubleRow
    (2 k-tiles per instruction, 0.5 cyc/row): G, Q = G B^T and the colsum
    drop from ~9.5us to ~2.4us of PE time. G is scaled by 1/256 in the
    psum->sbuf copy so Q/r stay inside fp8 range; the rsqrt activation's
    input scale of 256 compensates exactly. The norm chain only shapes the
    per-column scale s (~0.3% error budget); the main GEMM stays bf16.
  - a (natural-layout A) arrives as fp8 (it only feeds G) and bt8 is cast
    from bt on ACT instead of DMA'd: input bytes drop 3.54 -> 2.95 MB.
  - The s broadcast and the bts muls run on the otherwise-idle Pool engine
    (gpsimd); PE loses the old broadcast matmul, DVE loses the bts muls.
  - Panel drains are 1024 wide (two bank-aligned 512 matmul halves, one
    copy) to halve the per-op PSUM/SBUF access-latency tax, split ~6:7
    DVE:ACT to balance DVE's r muls against ACT's casts/rsqrts.
"""
import numpy as np

B, H, W, C = 8, 48, 48, 256
HW = H * W            # 2304
MT = HW // 128        # 18 m-tiles
T2 = MT // 2          # 9 paired m-tiles for DoubleRow G
CH_PIPE = [(0, 512), (512, 512), (1024, 512), (1536, 512), (2048, 256)]
CH_MM = [(0, 1024), (1024, 1024), (2048, 256)]

_CACHE = {}


def _build(reps=1):
    import concourse.bacc as bacc
    import concourse.mybir as mybir
    import concourse.tile as tile

    dt = mybir.dt
    f32 = dt.float32
    bf16 = dt.bfloat16
    fp8 = dt.float8e4
    DR = mybir.MatmulPerfMode.DoubleRow

    nc = bacc.Bacc(None, target_bir_lowering=False, debug=False)
    # Partition-major swizzled inputs (see marshal_inputs):
    #   a8 [p, (2*t+j)*C+c] = A[(2*t+j)*128+p, c]     (fp8, DoubleRow pairs)
    #   at [p, h*HW+n]      = A[n, h*128+p]           (A^T, GEMM lhsT source)
    #   bt [p, h*HW+n]      = B[n, h*128+p]           (B^T, chain + GEMM rhs)
    a8_dram = nc.dram_tensor("a8", [128, MT * C], fp8, kind="ExternalInput")
    at_dram = nc.dram_tensor("at", [128, 2 * HW], bf16, kind="ExternalInput")
    bt_dram = nc.dram_tensor("bt", [128, 2 * HW], bf16, kind="ExternalInput")
    o_dram = nc.dram_tensor("out", [HW, HW], bf16, kind="ExternalOutput")
    o_r = o_dram[:, :].rearrange("(t p) n -> p t n", p=128)

    with tile.TileContext(nc) as tc, nc.allow_low_precision(
            reason="bf16/fp8 pipeline is intentional; l2 tolerance is 2e-2"):
        consts = tc.alloc_tile_pool(name="consts", bufs=1)
        # dual-fp8 ldweights needs >=32 weight columns: broadcast the
        # colsum into 32 identical psum rows and read row 0.
        ones_f = consts.tile([128, 2, 32], f32)
        nc.vector.memset(ones_f, 1.0)
        ones8 = consts.tile([128, 2, 32], fp8)
        nc.vector.tensor_copy(ones8, ones_f)

        inp = tc.alloc_tile_pool(name="inp", bufs=2)
        sca = tc.alloc_tile_pool(name="sca", bufs=2)
        scr = tc.alloc_tile_pool(name="scr", bufs=3)
        panels = tc.alloc_tile_pool(name="panels", bufs=8)
        # PSUM: 3 x [128,1024] f32 GEMM slots (2 banks each) + 2 chain slots
        # [128,512] (pg / pq pair / pp rotate through them) = 8 banks.
        ps_gq = tc.alloc_tile_pool(name="ps_gq", bufs=2, space="PSUM")
        ps_mm = tc.alloc_tile_pool(name="ps_mm", bufs=3, space="PSUM")

        NCP = len(CH_PIPE)
        NCM = len(CH_MM)

        def make_tiles():
            a8 = inp.tile([128, T2, 2, C], fp8, tag="a8", name="a8")
            at = inp.tile([128, 2 * HW], bf16, tag="at", name="at")
            bt = inp.tile([128, 2 * HW], bf16, tag="bt", name="bt")
            bt8 = inp.tile([128, 2, HW], fp8, tag="bt8", name="bt8")
            g8 = sca.tile([128, 2, C], fp8, tag="g", name="g8")
            s_bf = sca.tile([1, HW], bf16, tag="s", name="s")
            sbc = sca.tile([128, HW], bf16, tag="sbc", name="sbc")
            bts = sca.tile([128, 2 * HW], bf16, tag="bts", name="bts")
            return dict(a8=a8, at=at, bt=bt, bt8=bt8, g8=g8, s_bf=s_bf,
                        sbc=sbc, bts=bts)

        def emit_dmas(tl):
            # a8 first: G gates the chain's critical path. Each partition
            # line >= 2KB keeps DMA at line rate (a8 4608B, halves 4608B).
            nc.sync.dma_start(out=tl["a8"], in_=a8_dram[:, :].rearrange(
                "p (t j c) -> p t j c", t=T2, j=2))
            for h in (0, 1):
                nc.sync.dma_start(out=tl["bt"][:, h * HW:(h + 1) * HW],
                                  in_=bt_dram[:, h * HW:(h + 1) * HW])
            nc.sync.dma_start(out=tl["at"], in_=at_dram[:, :])

        def emit_g(tl):
            # G = A^T A ([256,256]) in fp8 DoubleRow: 2 m-tiles per matmul,
            # both c-halves computed directly (Q's lhsT reads G[k, m] and G
            # is symmetric, so no transpose is needed either way).
            # g8[p, j, c'] = G[j*128+p, c'] / 1024: e4m3 tops out at 240 (inf
            # above), and r = B^T o (G/1024 B^T) must stay finite even on
            # tail columns (|r| <~ 60 at this scale).
            a8, g8 = tl["a8"], tl["g8"]
            pg = ps_gq.tile([128, 512], f32, tag="pgq", name="pg")
            for j2 in (0, 1):
                for t in range(T2):
                    nc.tensor.matmul(
                        pg[:, j2 * C:(j2 + 1) * C],
                        a8[:, t, :, j2 * 128:(j2 + 1) * 128],
                        a8[:, t, :, :],
                        start=(t == 0), stop=(t == T2 - 1),
                        perf_mode=DR)
            for j in (0, 1):
                nc.scalar.activation(
                    g8[:, j, :], pg[:, j * C:(j + 1) * C],
                    mybir.ActivationFunctionType.Copy, scale=1.0 / 1024)
            # bt8: fp8 cast of B^T for the DoubleRow chain (saves its DMA)
            for h in (0, 1):
                nc.scalar.copy(tl["bt8"][:, h, :],
                               tl["bt"][:, h * HW:(h + 1) * HW])

        def chunk_pipe(tl, ci):
            # pq_j = (G/256) B^T (one DoubleRow matmul per c-half);
            # r[:,j,:] = B^T o pq_j (fp8); pen2 = DoubleRow-colsum(r);
            # s = rsqrt(1024 * pen2/1024); sbc = broadcast s (Pool);
            # bts = B^T * s (Pool)
            bt, bt8, g8 = tl["bt"], tl["bt8"], tl["g8"]
            s_bf, sbc, bts = tl["s_bf"], tl["sbc"], tl["bts"]
            n0, cw = CH_PIPE[ci]
            r = scr.tile([128, 2, cw], fp8, tag=f"r{ci % 3}", name="r")
            for j in (0, 1):
                pq = ps_gq.tile([128, 512], f32, tag="pgq", name="pq")
                nc.tensor.matmul(
                    pq[:, :cw],
                    g8[:, :, j * 128:(j + 1) * 128],
                    bt8[:, :, n0:n0 + cw],
                    start=True, stop=True, perf_mode=DR)
                nc.vector.tensor_mul(
                    r[:, j, :], bt[:, j * HW + n0:j * HW + n0 + cw],
                    pq[:, :cw])
            pp = ps_gq.tile([128, 512], f32, tag="pgq", name="pp")
            nc.tensor.matmul(pp[0:32, :cw], ones8, r[:, :, :],
                             start=True, stop=True, perf_mode=DR)
            # s = 1/sqrt(pen2): ACT rsqrt, input scale 1024 undoing the g8
            # scaling exactly (pen2 >= 0 so |x| = x).
            nc.scalar.activation(
                s_bf[:, n0:n0 + cw], pp[0:1, :cw],
                mybir.ActivationFunctionType.Abs_reciprocal_sqrt, scale=1024.0)
            nc.gpsimd.partition_broadcast(sbc[:, n0:n0 + cw],
                                          s_bf[0:1, n0:n0 + cw])
            # all-bf16 muls hit DVE's 2x mode (0.52 ns/col)
            for h in (0, 1):
                nc.vector.tensor_mul(
                    bts[:, h * HW + n0:h * HW + n0 + cw],
                    bt[:, h * HW + n0:h * HW + n0 + cw],
                    sbc[:, n0:n0 + cw])

        # Software pipeline across reps: rep r+1's input DMAs are issued at
        # wavefront step 4 of rep r (the in-order HWDGE queue reaches them
        # mid-body instead of after all of rep r's panel DMAs), and rep r+1's
        # G matmuls are emitted at step 12 (the PE FIFO reaches them when
        # a8 has long landed, so G runs gap-free inside rep r's stream).
        tiles = make_tiles()
        emit_dmas(tiles)
        emit_g(tiles)
        chunk_pipe(tiles, 0)
        chunk_pipe(tiles, 1)
        pipes_pre = False
        for _rep in range(reps):
            tl = tiles
            nxt = None

            # main GEMM on a diagonal wavefront: step k emits (mt, ci) with
            # mt = k - ci, so program order (= PE FIFO order) only ever needs
            # GEMM chunk ci ~k*2us after GEMM start; the chain chunks feeding
            # it are emitted one step ahead (rep 0 only -- later reps' chains
            # were pre-run inside the previous body, so their wavefronts
            # start with every bts chunk ready).
            at, bts = tl["at"], tl["bts"]
            panel_by_mt = {}
            for k in range(MT + NCM - 1):
                if not pipes_pre:
                    if k == 0:
                        chunk_pipe(tl, 2)
                        chunk_pipe(tl, 3)
                    elif k == 1:
                        chunk_pipe(tl, 4)
                if k == 3 and _rep + 1 < reps:
                    nxt = make_tiles()
                    emit_dmas(nxt)
                if k == 8 and nxt is not None:
                    emit_g(nxt)
                if nxt is not None and 10 <= k < 10 + NCP:
                    chunk_pipe(nxt, k - 10)
                for ci in range(NCM):
                    mt = k - ci
                    if not (0 <= mt < MT):
                        continue
                    n0, cw = CH_MM[ci]
                    if ci == 0:
                        panel_by_mt[mt] = panels.tile([128, HW], bf16,
                                                      tag="panel",
                                                      name="panel")
                    panel = panel_by_mt[mt]
                    # 3 pm slots (2 banks each): the wavefront keeps ~2 in
                    # flight; matmuls write bank-aligned 512 halves, the
                    # drain covers the whole 1024 in one op.
                    pm = ps_mm.tile([128, cw], f32, tag="pm", name="pm")
                    for q0 in range(0, cw, 512):
                        qw = min(512, cw - q0)
                        for h in (0, 1):
                            nc.tensor.matmul(
                                pm[:, q0:q0 + qw],
                                at[:, h * HW + mt * 128:h * HW + (mt + 1) * 128],
                                bts[:, h * HW + n0 + q0:h * HW + n0 + q0 + qw],
                                start=(h == 0), stop=(h == 1))
                    # ~6/13 of panel drains on DVE, rest on ACT: DVE also
                    # carries the chain's r muls, ACT the casts and rsqrts.
                    cp = (nc.vector.tensor_copy if (mt * 3 + ci) % 13 < 6
                          else nc.scalar.copy)
                    cp(panel[:, n0:n0 + cw], pm[:, :])
                    # split the panel store: the first piece fires two steps
                    # early, smoothing the write stream; both pieces keep
                    # partition lines >= 2KB for DMA line rate
                    if ci == 0:
                        nc.sync.dma_start(out=o_r[:, mt, :1024],
                                          in_=panel[:, :1024])
                    elif ci == NCM - 1:
                        nc.sync.dma_start(out=o_r[:, mt, 1024:],
                                          in_=panel[:, 1024:])
            if nxt is not None:
                tiles = nxt
                pipes_pre = True

        for pool in (ps_mm, ps_gq,
                     panels, scr, sca, inp, consts):
            pool.release()
    nc.finalize()
    return nc


def _get_nc(reps=1):
    key = ("nc", reps)
    if key not in _CACHE:
        _CACHE[key] = _build(reps)
    return _CACHE[key]


def marshal_inputs(feature_A, feature_B):
    """Full f32 inputs -> per-core partition-major bf16/fp8 arrays."""
    import ml_dtypes
    bf = ml_dtypes.bfloat16
    f8 = ml_dtypes.float8_e4m3
    fa = np.asarray(feature_A, dtype=np.float32).reshape(B, HW, C)
    fb = np.asarray(feature_B, dtype=np.float32).reshape(B, HW, C)
    # a8[b, p, (2*t+j)*C+c] = A[b, (2*t+j)*128+p, c]
    a8 = np.ascontiguousarray(
        fa.astype(f8).reshape(B, MT, 128, C).transpose(0, 2, 1, 3)
    ).reshape(B, 128, MT * C)
    # at[b, p, h*HW+n] = A[b, n, h*128+p]
    at_sw = np.ascontiguousarray(
        fa.astype(bf).reshape(B, HW, 2, 128).transpose(0, 3, 2, 1)
    ).reshape(B, 128, 2 * HW)
    bt_sw = np.ascontiguousarray(
        fb.astype(bf).reshape(B, HW, 2, 128).transpose(0, 3, 2, 1)
    ).reshape(B, 128, 2 * HW)
    return a8, at_sw, bt_sw


def run(feature_A, feature_B, trace=False):
    from concourse.bass_utils import run_bass_kernel_spmd

    nc = _get_nc()
    a8, at_sw, bt_sw = marshal_inputs(feature_A, feature_B)
    in_maps = [{"a8": a8[i], "at": at_sw[i], "bt": bt_sw[i]}
               for i in range(B)]
    res = run_bass_kernel_spmd(nc, in_maps, list(range(B)), trace=trace)
    out = np.stack([res.results[i]["out"].astype(np.float32)
                    for i in range(B)])
    return out.reshape(B, H, W, H, W), res


def kernel(feature_A, feature_B):
    out, _ = run(feature_A, feature_B)
    return out
